# revision 1
# baseline (speedup 1.0000x reference)
"""Trainium2 Bass kernel for the Cross_Attention module.

Math (per batch b, per output stream):
  f1 = Wf1 @ x + bf1         [D, N]   (from x, both streams)
  f2 = Wf2 @ y + bf2         [D, N]   (from y, both streams)
  g  = Wg  @ z + bg          [D, N]   (z = x for the x_out stream, y for y_out)
  h  = Wh  @ x + bh          [D, N]   (always from x)
  A_a[i, j] = softmax_j(f_a[:, i] . g[:, j])        a in {1, 2}
  out = z + ga * (Wv1 @ (h A_1^T) + bv1) + gb * (Wv2 @ (h A_2^T) + bv2)

Sharding: 8 cores = 4 batches x 2 streams (x_out / y_out). No collectives.

Device algorithm (per core):
  - logits computed TRANSPOSED: LT[j, i] = sum_d g[d, j] f[d, i] so that the
    softmax reduction axis j lands on PSUM partitions. The two attentions
    share the stationary g tile and run as concurrent row-tiled matmuls
    (f1 in partitions 0:64, f2 in 64:128).
  - E = exp(LT - 40)  (constant shift; |logits| << 40+88 so exp is safe, and
    softmax is shift-invariant so the result is exact).
  - num[., i] = [hT | ones]^T @ E: one matmul per j-tile accumulates both the
    numerator (rows 0..63) and the softmax denominator (row 64).
  - EA = num[0:64] * (1/num[64]) broadcast via a K=1 ones matmul.
  - out = z + (ga*Wv1) @ EA1 + (gb*Wv2) @ EA2 + (ga*bv1 + gb*bv2).
Matmuls run as float32r (full-rate on TRN2 when the moving dim >= 256).
The input projections are software-pipelined into attention i-block 0 so
compute starts as soon as the first input chunks arrive from HBM.
"""

import numpy as np

import concourse.bass as bass
import concourse.bacc as bacc
import concourse.mybir as mybir
import concourse.tile as tile
from concourse.masks import make_identity

BS = 4
C = 512
D = 64
H = W = 48
N = H * W          # 2304
P = 128
NK = C // P        # 4 contraction tiles for the projections
NCT = C // P       # 4 output channel tiles
NJT = N // P       # 18 j tiles
IBLK = 512
IBLOCKS = [(0, 512), (512, 512), (1024, 512), (1536, 512), (2048, 256)]
NNB = len(IBLOCKS)
SHIFT = 40.0

F32 = mybir.dt.float32
F32R = mybir.dt.float32r
BF16 = mybir.dt.bfloat16
AF = mybir.ActivationFunctionType
OP = mybir.AluOpType


def _bcast_dram(handle, parts, free):
    """AP reading `free` leading elements of a DRAM tensor, replicated to
    `parts` partitions (partition stride 0)."""
    return bass.AP(tensor=handle, offset=0, ap=[[0, parts], [1, free]])


def build_program():
    nc = bacc.Bacc("TRN2", target_bir_lowering=False)

    xin = nc.dram_tensor("xin", [C, N], BF16, kind="ExternalInput")
    yin = nc.dram_tensor("yin", [C, N], BF16, kind="ExternalInput")
    zin = nc.dram_tensor("zin", [C, N], F32, kind="ExternalInput")
    # weights arrive pre-transposed ([C, D] / [D, C]) and the value weights
    # pre-scaled by their gates; cv = ga*bv1 + gb*bv2 (host-side marshalling)
    WfT1 = nc.dram_tensor("WfT1", [C, D], BF16, kind="ExternalInput")
    WfT2 = nc.dram_tensor("WfT2", [C, D], BF16, kind="ExternalInput")
    WgT = nc.dram_tensor("WgT", [C, D], F32, kind="ExternalInput")
    WhT = nc.dram_tensor("WhT", [C, D], BF16, kind="ExternalInput")
    WvT1 = nc.dram_tensor("WvT1", [D, C], F32, kind="ExternalInput")
    WvT2 = nc.dram_tensor("WvT2", [D, C], F32, kind="ExternalInput")
    bf1 = nc.dram_tensor("bf1", [D, 1], F32, kind="ExternalInput")
    bf2 = nc.dram_tensor("bf2", [D, 1], F32, kind="ExternalInput")
    bg = nc.dram_tensor("bg", [D, 1], F32, kind="ExternalInput")
    bh = nc.dram_tensor("bh", [D, 1], F32, kind="ExternalInput")
    cv = nc.dram_tensor("cv", [C, 1], F32, kind="ExternalInput")
    out = nc.dram_tensor("out", [C, N], F32, kind="ExternalOutput")

    xin_r = xin.rearrange("(co ci) n -> ci co n", ci=P)
    yin_r = yin.rearrange("(co ci) n -> ci co n", ci=P)
    zin_r = zin.rearrange("(co ci) n -> ci co n", ci=P)
    out_r = out.rearrange("(co ci) n -> ci co n", ci=P)

    with tile.TileContext(nc) as tc:
        with (
            tc.tile_pool(name="persist", bufs=1) as persist,
            tc.tile_pool(name="scratch", bufs=2, space="PSUM") as scratch,
            tc.tile_pool(name="ltp", bufs=2, space="PSUM") as ltp,
            tc.tile_pool(name="nump", bufs=2, space="PSUM") as nump,
            tc.tile_pool(name="ebuf", bufs=3) as ebuf,
            tc.tile_pool(name="eap", bufs=2) as eap,
            tc.tile_pool(name="rcpp", bufs=2) as rcpp,
            tc.tile_pool(name="osbp", bufs=3) as osbp,
        ):
            xin_sb = persist.tile([P, NK, N], BF16)
            yin_sb = persist.tile([P, NK, N], BF16)
            zin_sb = persist.tile([P, NK, N], F32R)
            h_sb = persist.tile([D, N], F32)
            # block-0 input chunks go first so the first projections can
            # start while the small/weight DMAs stream behind them
            i0, w = IBLOCKS[0]
            sl0 = slice(i0, i0 + w)
            nc.sync.dma_start(out=zin_sb[:, :, sl0],
                              in_=zin_r[:, :, sl0].bitcast(F32R))
            nc.sync.dma_start(out=xin_sb[:, :, sl0], in_=xin_r[:, :, sl0])
            nc.sync.dma_start(out=yin_sb[:, :, sl0], in_=yin_r[:, :, sl0])

            # ---------------- constants / small inputs ----------------
            # (small DMAs ride the gpsimd queue so the big input streams own
            # the sync queue from t=0)
            identity = persist.tile([P, P], F32)
            make_identity(nc, identity)
            onesF = persist.tile([P, 1], F32)
            nc.vector.memset(onesF, 1.0)
            ones_row = persist.tile([P, D], F32R)
            nc.vector.tensor_copy(out=ones_row,
                                  in_=onesF[:, 0:1].broadcast_to([P, D]))
            shiftb = persist.tile([P, 1], F32)
            nc.vector.memset(shiftb, -SHIFT)

            bfa_sb = persist.tile([D, 1], F32)
            nc.sync.dma_start(out=bfa_sb, in_=bf1[:, :])
            bfb_sb = persist.tile([D, 1], F32)
            nc.sync.dma_start(out=bfb_sb, in_=bf2[:, :])
            bgx_sb = persist.tile([D, 1], F32)
            nc.sync.dma_start(out=bgx_sb, in_=bg[:, :])
            bh_sb = persist.tile([D, 1], F32)
            nc.sync.dma_start(out=bh_sb, in_=bh[:, :])

            cvec = persist.tile([P, NCT, 1], F32)
            nc.sync.dma_start(
                out=cvec, in_=cv.rearrange("(ct ci) one -> ci ct one", ci=P))

            # ---------------- persistent activations ----------------
            f12_sb = persist.tile([P, N], F32R)   # rows 0:64 f1, rows 64:128 f2
            g2x_sb = persist.tile([P, N], F32R)   # g replicated in both halves
            hT_sb = persist.tile([P, NJT, D + 1], F32R)
            # ones column (index D); written via DVE so it rounds to f32r
            nc.vector.tensor_copy(
                out=hT_sb[:, :, D],
                in_=onesF[:, 0:1].broadcast_to([P, NJT]))

            Wf1T_sb = persist.tile([P, NK, D], BF16)
            nc.sync.dma_start(
                out=Wf1T_sb, in_=WfT1.rearrange("(k ci) d -> ci k d", ci=P))
            Wf2T_sb = persist.tile([P, NK, D], BF16)
            nc.sync.dma_start(
                out=Wf2T_sb, in_=WfT2.rearrange("(k ci) d -> ci k d", ci=P))
            WgT_sb = persist.tile([P, NK, D], F32R)
            nc.sync.dma_start(
                out=WgT_sb,
                in_=WgT.rearrange("(k ci) d -> ci k d", ci=P).bitcast(F32R))
            WhT_sb = persist.tile([P, NK, D], BF16)
            nc.sync.dma_start(
                out=WhT_sb, in_=WhT.rearrange("(k ci) d -> ci k d", ci=P))
            Wv1T_sb = persist.tile([D, NCT, P], F32R)
            nc.sync.dma_start(
                out=Wv1T_sb,
                in_=WvT1.rearrange("d (ct ci) -> d ct ci", ci=P).bitcast(F32R))
            Wv2T_sb = persist.tile([D, NCT, P], F32R)
            nc.sync.dma_start(
                out=Wv2T_sb,
                in_=WvT2.rearrange("d (ct ci) -> d ct ci", ci=P).bitcast(F32R))


            # remaining input chunks stream behind the weights
            for i0, w in IBLOCKS[1:]:
                sl = slice(i0, i0 + w)
                nc.sync.dma_start(out=zin_sb[:, :, sl],
                                  in_=zin_r[:, :, sl].bitcast(F32R))
                nc.sync.dma_start(out=xin_sb[:, :, sl], in_=xin_r[:, :, sl])
                nc.sync.dma_start(out=yin_sb[:, :, sl], in_=yin_r[:, :, sl])

            def emit_proj(nb):
                """Projection chains + hT transposes for column block nb."""
                i0, w = IBLOCKS[nb]
                sl = slice(i0, i0 + w)
                # chain order (g, f1, f2, h): the logits for this column
                # block need g and f first; hT is consumed a beat later
                # chain order (g, f1, f2, h), each chain contiguous so an
                # input chunk that is still in flight only stalls its own
                # chain (the logits need g and f first; hT a beat later)
                pf1 = scratch.tile([P, IBLK], F32, tag="sc", name="pf1")
                for k in range(NK):
                    nc.tensor.matmul(
                        pf1[0:D, :w], lhsT=WgT_sb[:, k, :],
                        rhs=zin_sb[:, k, sl], start=(k == 0), stop=(k == NK - 1))
                # g lands in both partition halves (partition-shifted copy)
                nc.vector.tensor_scalar_add(
                    out=g2x_sb[0:D, sl], in0=pf1[0:D, :w], scalar1=bgx_sb)
                nc.vector.tensor_scalar_add(
                    out=g2x_sb[D:P, sl], in0=pf1[0:D, :w], scalar1=bgx_sb)
                pf3 = scratch.tile([P, IBLK], F32, tag="sc", name="pf3")
                for k in range(NK):
                    nc.tensor.matmul(
                        pf3[0:D, :w], lhsT=Wf1T_sb[:, k, :],
                        rhs=xin_sb[:, k, sl], start=(k == 0), stop=(k == NK - 1))
                nc.vector.tensor_scalar_add(
                    out=f12_sb[0:D, sl], in0=pf3[0:D, :w], scalar1=bfa_sb)
                pf4 = scratch.tile([P, IBLK], F32, tag="sc", name="pf4")
                for k in range(NK):
                    nc.tensor.matmul(
                        pf4[0:D, :w], lhsT=Wf2T_sb[:, k, :],
                        rhs=yin_sb[:, k, sl], start=(k == 0), stop=(k == NK - 1))
                nc.vector.tensor_scalar_add(
                    out=f12_sb[D:P, sl], in0=pf4[0:D, :w], scalar1=bfb_sb)
                pf2 = scratch.tile([P, IBLK], F32, tag="sc", name="pf2")
                for k in range(NK):
                    nc.tensor.matmul(
                        pf2[0:D, :w], lhsT=WhT_sb[:, k, :],
                        rhs=xin_sb[:, k, sl], start=(k == 0), stop=(k == NK - 1))
                nc.vector.tensor_scalar_add(
                    out=h_sb[:, sl], in0=pf2[0:D, :w], scalar1=bh_sb)
                for jt in range(i0 // P, (i0 + w) // P):
                    pT = scratch.tile([P, IBLK], F32, tag="sc", name="pTh")
                    nc.tensor.transpose(
                        pT[:, 0:D], h_sb[0:D, jt * P:(jt + 1) * P],
                        identity[0:D, 0:D])
                    nc.vector.tensor_copy(
                        out=hT_sb[:, jt, 0:D], in_=pT[:, 0:D])

            emit_proj(0)

            # ---------------- attention main loop ----------------
            def emit_out(ea1, ea2, i0, w):
                for ct in range(NCT):
                    ops = scratch.tile([P, IBLK], F32, tag="sc", name="ops")
                    nc.tensor.matmul(
                        ops[:, :w], lhsT=Wv1T_sb[:, ct, :],
                        rhs=ea1[:, :w], start=True, stop=False)
                    nc.tensor.matmul(
                        ops[:, :w], lhsT=Wv2T_sb[:, ct, :],
                        rhs=ea2[:, :w], start=False, stop=True)
                    osb = osbp.tile([P, IBLK], F32, tag="osb", name="osb")
                    nc.vector.scalar_tensor_tensor(
                        out=osb[:, :w], in0=ops[:, :w],
                        scalar=cvec[:, ct, :],
                        in1=zin_sb[:, ct, i0:i0 + w],
                        op0=OP.add, op1=OP.add)
                    nc.sync.dma_start(
                        out=out_r[:, ct, i0:i0 + w], in_=osb[:, :w])

            pending = None
            for ib, (i0, w) in enumerate(IBLOCKS):
                isl = slice(i0, i0 + w)
                num1 = nump.tile([D + 1, IBLK], F32, tag="num", name="num1")
                num2 = nump.tile([D + 1, IBLK], F32, tag="num", name="num2")
                prev_et = None
                for jt in range(NJT + 1):
                    if ib == 0 and jt % 4 == 1 and jt // 4 + 1 < NNB:
                        emit_proj(jt // 4 + 1)
                    if jt < NJT:
                        j0 = jt * P
                        lt = ltp.tile([P, 2, IBLK], F32, tag="lt", name="lt")
                        nc.tensor.matmul(
                            lt[:, 0, :w], lhsT=g2x_sb[0:D, j0:j0 + P],
                            rhs=f12_sb[0:D, isl],
                            start=True, stop=True, tile_position=(0, 0))
                        nc.tensor.matmul(
                            lt[:, 1, :w], lhsT=g2x_sb[D:P, j0:j0 + P],
                            rhs=f12_sb[D:P, isl],
                            start=True, stop=True, tile_position=(64, 0))
                        et = ebuf.tile([P, 2, IBLK], F32R, tag="et", name="et")
                        nc.scalar.activation(
                            out=et[:, :, :w], in_=lt[:, :, :w],
                            func=AF.Exp, bias=shiftb[:, 0:1], scale=1.0)
                    if jt > 0:
                        pj = jt - 1
                        st, sp = (pj == 0), (pj == NJT - 1)
                        nc.tensor.matmul(
                            num1[:, :w], lhsT=hT_sb[:, pj, :],
                            rhs=prev_et[:, 0, :w], start=st, stop=sp)
                        nc.tensor.matmul(
                            num2[:, :w], lhsT=hT_sb[:, pj, :],
                            rhs=prev_et[:, 1, :w], start=st, stop=sp)
                    if jt < NJT:
                        prev_et = et
                    if jt == 3 and pending is not None:
                        emit_out(*pending)
                        pending = None
                eas = []
                for num in (num1, num2):
                    rcp = rcpp.tile([1, IBLK], F32R, tag="rcp", name="rcp")
                    with nc.allow_low_precision(
                            reason="softmax denominator reciprocal in f32r"):
                        # partition-shifted read: s lives in num row D
                        nc.vector.reciprocal(rcp[0:1, :w], num[D:D + 1, :w])
                    rb = scratch.tile([P, IBLK], F32, tag="sc", name="rb")
                    # broadcast 1/s to 64 partitions: K=1 ones matmul
                    nc.tensor.matmul(
                        rb[0:D, :w], lhsT=ones_row[0:1, 0:D],
                        rhs=rcp[0:1, :w], start=True, stop=True)
                    ea = eap.tile([D, IBLK], F32R, tag="ea", name="ea")
                    nc.vector.tensor_copy(out=ea[:, :w], in_=num[0:D, :w])
                    nc.vector.tensor_mul(ea[:, :w], ea[:, :w], rb[0:D, :w])
                    eas.append(ea)
                pending = (eas[0], eas[1], i0, w)
            emit_out(*pending)

    nc.compile()
    return nc


_NC_CACHE = None


def _get_nc():
    global _NC_CACHE
    if _NC_CACHE is None:
        _NC_CACHE = build_program()
    return _NC_CACHE


def _run(inputs, trace=False, trace_cores=None):
    from concourse.bass_utils import run_bass_kernel_spmd

    import ml_dtypes
    g = {k: np.ascontiguousarray(np.asarray(v, dtype=np.float32))
         for k, v in inputs.items()}
    x = g["x"].reshape(BS, C, N)
    y = g["y"].reshape(BS, C, N)
    x16 = np.ascontiguousarray(x.astype(ml_dtypes.bfloat16))
    y16 = np.ascontiguousarray(y.astype(ml_dtypes.bfloat16))

    def core_inputs(b, s):
        def sel(a0, a1):
            return a0 if s == 0 else a1

        gate1 = float(np.asarray(sel(g["alpha"], g["gamma"])).reshape(-1)[0])
        gate2 = float(np.asarray(sel(g["beta"], g["sigma"])).reshape(-1)[0])
        return {
            "xin": np.ascontiguousarray(x16[b]),
            "yin": np.ascontiguousarray(y16[b]),
            "zin": np.ascontiguousarray(sel(x, y)[b]),
            "WfT1": np.ascontiguousarray(
                g["Wf1"].T.astype(ml_dtypes.bfloat16)),
            "WfT2": np.ascontiguousarray(
                g["Wf2"].T.astype(ml_dtypes.bfloat16)),
            "WgT": np.ascontiguousarray(sel(g["Wg1"], g["Wg2"]).T),
            "WhT": np.ascontiguousarray(
                sel(g["Wh1"], g["Wh2"]).T.astype(ml_dtypes.bfloat16)),
            "WvT1": np.ascontiguousarray(
                gate1 * sel(g["Wv11"], g["Wv12"]).T),
            "WvT2": np.ascontiguousarray(
                gate2 * sel(g["Wv21"], g["Wv22"]).T),
            "bf1": g["bf1"].reshape(D, 1), "bf2": g["bf2"].reshape(D, 1),
            "bg": sel(g["bg1"], g["bg2"]).reshape(D, 1),
            "bh": sel(g["bh1"], g["bh2"]).reshape(D, 1),
            "cv": (gate1 * sel(g["bv11"], g["bv12"])
                   + gate2 * sel(g["bv21"], g["bv22"])).reshape(C, 1),
        }

    in_maps = [core_inputs(core // 2, core % 2) for core in range(8)]
    res = run_bass_kernel_spmd(
        _get_nc(), in_maps, core_ids=list(range(8)), trace=trace,
        trace_cores=trace_cores)
    outs = [r["out"] for r in res.results]
    x_out = np.stack([outs[2 * b] for b in range(BS)]).reshape(BS, C, H, W)
    y_out = np.stack([outs[2 * b + 1] for b in range(BS)]).reshape(BS, C, H, W)
    return (x_out, y_out), res


def kernel(**inputs):
    out, _ = _run(inputs)
    return out



# revision 14
# speedup vs baseline: 1.1144x; 1.1144x over previous
"""Trainium2 Bass kernel for the Cross_Attention module.

Math (per batch b, per output stream):
  f1 = Wf1 @ x + bf1         [D, N]   (from x, both streams)
  f2 = Wf2 @ y + bf2         [D, N]   (from y, both streams)
  g  = Wg  @ z + bg          [D, N]   (z = x for the x_out stream, y for y_out)
  h  = Wh  @ x + bh          [D, N]   (always from x)
  A_a[i, j] = softmax_j(f_a[:, i] . g[:, j])        a in {1, 2}
  out = z + ga * (Wv1 @ (h A_1^T) + bv1) + gb * (Wv2 @ (h A_2^T) + bv2)

Sharding: 8 cores = 4 batches x 2 streams (x_out / y_out). No collectives.

Device algorithm (per core):
  - logits computed TRANSPOSED: LT[j, i] = sum_d g[d, j] f[d, i] so that the
    softmax reduction axis j lands on PSUM partitions. The two attentions
    share the stationary g tile and run as row-tiled bf16 matmuls
    (f1 in partitions 0:64, f2 in 64:128).
  - E = exp(LT - 40)  (constant shift; |logits| << 40+88 so exp is safe, and
    softmax is shift-invariant so the result is exact).
  - num[., i] = [hT | ones]^T @ E: one matmul per j-tile accumulates both the
    numerator (rows 0..63) and the softmax denominator (row 64).
  - EA = num[0:64] * (1/num[64]) broadcast via a K=2 selector matmul that
    serves both attentions at once.
  - out = z + [ga*Wv1 | gb*Wv2] @ [EA1; EA2] + (ga*bv1 + gb*bv2): the two
    value GEMMs are one K=128 matmul against host-stacked weights.
Projection chains are merged ([Wf1|Wh] is one K=128-wide stationary) and
pipelined in 256-column chunks against the input DMA stream, so compute
starts ~3us in and the attention loop is paced by the Activation engine
(the exp of 2*N^2 logits is the hard floor of this problem).
"""

import numpy as np

import concourse.bass as bass
import concourse.bacc as bacc
import concourse.mybir as mybir
import concourse.tile as tile
from concourse.masks import make_identity

BS = 4
C = 512
D = 64
H = W = 48
N = H * W          # 2304
P = 128
NK = C // P        # 4 contraction tiles for the projections
NCT = C // P       # 4 output channel tiles
NJT = N // P       # 18 j tiles
IBLK = 512
IBLOCKS = [(0, 512), (512, 512), (1024, 512), (1536, 512), (2048, 256)]
CHUNK = 256        # projection / input streaming chunk (columns)
NCH = N // CHUNK   # 9
SHIFT = 40.0

F32 = mybir.dt.float32
F32R = mybir.dt.float32r
BF16 = mybir.dt.bfloat16
AF = mybir.ActivationFunctionType
OP = mybir.AluOpType


def build_program():
    nc = bacc.Bacc("TRN2", target_bir_lowering=False)

    xin = nc.dram_tensor("xin", [C, N], BF16, kind="ExternalInput")
    yin = nc.dram_tensor("yin", [C, N], BF16, kind="ExternalInput")
    zin = nc.dram_tensor("zin", [C, N], BF16, kind="ExternalInput")
    # host-marshalled weights: Wf1h = [Wf1.T | Wh.T] (f1+h share one chain),
    # WvS = [ga*Wv1.T ; gb*Wv2.T] stacked on the contraction dim,
    # smalls = biases + cv packed: col0=[bf1;bh] col1=[bf2;-] col2=[bg;-]
    # cols 4:8 = cv = ga*bv1 + gb*bv2 in (ci, ct) layout.
    Wf1h = nc.dram_tensor("Wf1h", [C, P], BF16, kind="ExternalInput")
    Wf2T = nc.dram_tensor("Wf2T", [C, D], BF16, kind="ExternalInput")
    WgT = nc.dram_tensor("WgT", [C, D], BF16, kind="ExternalInput")
    WvS = nc.dram_tensor("WvS", [P, C], F32, kind="ExternalInput")
    smalls = nc.dram_tensor("smalls", [P, 8], F32, kind="ExternalInput")
    sel = nc.dram_tensor("sel", [2, P], F32, kind="ExternalInput")
    out = nc.dram_tensor("out", [C, N], F32, kind="ExternalOutput")

    xin_r = xin.rearrange("(co ci) n -> ci co n", ci=P)
    yin_r = yin.rearrange("(co ci) n -> ci co n", ci=P)
    zin_r = zin.rearrange("(co ci) n -> ci co n", ci=P)
    out_r = out.rearrange("(co ci) n -> ci co n", ci=P)

    with tile.TileContext(nc) as tc:
        with (
            tc.tile_pool(name="persist", bufs=1) as persist,
            tc.tile_pool(name="scratch", bufs=2, space="PSUM") as scratch,
            tc.tile_pool(name="ltp", bufs=2, space="PSUM") as ltp,
            tc.tile_pool(name="nump", bufs=2, space="PSUM") as nump,
            tc.tile_pool(name="ebuf", bufs=3) as ebuf,
            tc.tile_pool(name="eap", bufs=2) as eap,
            tc.tile_pool(name="rcpp", bufs=2) as rcpp,
            tc.tile_pool(name="osbp", bufs=2) as osbp,
        ):
            xin_sb = persist.tile([P, NK, N], BF16)
            yin_sb = persist.tile([P, NK, N], BF16)
            zin_sb = persist.tile([P, NK, N], BF16)

            # weights ride the gpsimd (SWDGE) queue so they land in parallel
            # with the input stream on the sync queue
            WgT_sb = persist.tile([P, NK, D], BF16)
            nc.gpsimd.dma_start(
                out=WgT_sb, in_=WgT.rearrange("(k ci) d -> ci k d", ci=P))
            Wf1h_sb = persist.tile([P, NK, P], BF16)
            nc.gpsimd.dma_start(
                out=Wf1h_sb, in_=Wf1h.rearrange("(k ci) d -> ci k d", ci=P))
            Wf2T_sb = persist.tile([P, NK, D], BF16)
            nc.gpsimd.dma_start(
                out=Wf2T_sb, in_=Wf2T.rearrange("(k ci) d -> ci k d", ci=P))
            smalls_sb = persist.tile([P, 8], F32)
            nc.gpsimd.dma_start(out=smalls_sb, in_=smalls[:, :])
            WvS_sb = persist.tile([P, NCT, P], F32R)
            nc.gpsimd.dma_start(
                out=WvS_sb,
                in_=WvS.rearrange("d (ct ci) -> d ct ci", ci=P).bitcast(F32R))

            # inputs stream in CHUNK-col slices; z first (g chain gates the
            # logits), then x (f1+h), then y (f2)
            for ch in range(NCH):
                sl = slice(ch * CHUNK, (ch + 1) * CHUNK)
                nc.sync.dma_start(out=zin_sb[:, :, sl], in_=zin_r[:, :, sl])
                nc.sync.dma_start(out=xin_sb[:, :, sl], in_=xin_r[:, :, sl])
                nc.sync.dma_start(out=yin_sb[:, :, sl], in_=yin_r[:, :, sl])

            # ---------------- constants ----------------
            identity = persist.tile([P, P], F32)
            make_identity(nc, identity)
            onesF = persist.tile([P, 1], F32)
            nc.vector.memset(onesF, 1.0)
            shiftb = persist.tile([P, 1], F32)
            nc.vector.memset(shiftb, -SHIFT)
            # selector for the K=2 reciprocal broadcast: row0 -> parts 0:64,
            # row1 -> parts 64:128 (host-supplied 0/1 matrix)
            sel2 = persist.tile([2, P], F32R)
            nc.gpsimd.dma_start(out=sel2, in_=sel[:, :].bitcast(F32R))

            # ---------------- persistent activations ----------------
            f12_sb = persist.tile([P, N], BF16)   # rows 0:64 f1, 64:128 f2
            g2x_sb = persist.tile([P, N], BF16)   # g replicated in both halves
            h_sb = persist.tile([D, N], F32)
            hT_sb = persist.tile([P, NJT, D + 1], F32R)
            nc.vector.tensor_copy(
                out=hT_sb[:, :, D],
                in_=onesF[:, 0:1].broadcast_to([P, NJT]))

            def emit_proj(ch):
                """Projection chains + hT transposes for a 256-col chunk."""
                i0 = ch * CHUNK
                w = CHUNK
                sl = slice(i0, i0 + w)
                pg = scratch.tile([P, IBLK], F32, tag="sc", name="pg")
                for k in range(NK):
                    nc.tensor.matmul(
                        pg[0:D, :w], lhsT=WgT_sb[:, k, :],
                        rhs=zin_sb[:, k, sl], start=(k == 0), stop=(k == NK - 1))
                nc.vector.tensor_scalar_add(
                    out=g2x_sb[0:D, sl], in0=pg[0:D, :w],
                    scalar1=smalls_sb[0:D, 2:3])
                nc.vector.tensor_scalar_add(
                    out=g2x_sb[D:P, sl], in0=pg[0:D, :w],
                    scalar1=smalls_sb[0:D, 2:3])
                pf = scratch.tile([P, IBLK], F32, tag="sc", name="pf")
                for k in range(NK):
                    nc.tensor.matmul(
                        pf[:, :w], lhsT=Wf1h_sb[:, k, :],
                        rhs=xin_sb[:, k, sl], start=(k == 0), stop=(k == NK - 1))
                nc.vector.tensor_scalar_add(
                    out=f12_sb[0:D, sl], in0=pf[0:D, :w],
                    scalar1=smalls_sb[0:D, 0:1])
                nc.vector.tensor_scalar_add(
                    out=h_sb[:, sl], in0=pf[D:P, :w],
                    scalar1=smalls_sb[D:P, 0:1])
                pq = scratch.tile([P, IBLK], F32, tag="sc", name="pq")
                for k in range(NK):
                    nc.tensor.matmul(
                        pq[0:D, :w], lhsT=Wf2T_sb[:, k, :],
                        rhs=yin_sb[:, k, sl], start=(k == 0), stop=(k == NK - 1))
                nc.vector.tensor_scalar_add(
                    out=f12_sb[D:P, sl], in0=pq[0:D, :w],
                    scalar1=smalls_sb[0:D, 1:2])
                for jt in range(i0 // P, (i0 + w) // P):
                    pT = scratch.tile([P, IBLK], F32, tag="sc", name="pT")
                    nc.tensor.transpose(
                        pT[:, 0:D], h_sb[0:D, jt * P:(jt + 1) * P],
                        identity[0:D, 0:D])
                    nc.vector.tensor_copy(
                        out=hT_sb[:, jt, 0:D], in_=pT[:, 0:D])

            emit_proj(0)
            emit_proj(1)

            # ---------------- output projection ----------------
            def emit_out(ea, i0, w):
                osb = osbp.tile([P, NCT, IBLK], F32, tag="osb", name="osb")
                for ct in range(NCT):
                    # osb = z + cv first (waits only on the input stream),
                    # then += the value GEMM result
                    nc.vector.tensor_scalar_add(
                        out=osb[:, ct, :w], in0=zin_sb[:, ct, i0:i0 + w],
                        scalar1=smalls_sb[:, 4 + ct:5 + ct])
                    ops = scratch.tile([P, IBLK], F32, tag="sc", name="ops")
                    nc.tensor.matmul(
                        ops[:, :w], lhsT=WvS_sb[:, ct, :],
                        rhs=ea[:, :w], start=True, stop=True)
                    nc.vector.tensor_add(
                        osb[:, ct, :w], osb[:, ct, :w], ops[:, :w])
                nc.sync.dma_start(
                    out=out_r[:, :, i0:i0 + w], in_=osb[:, :, :w])

            # ---------------- attention main loop ----------------
            pending = None
            for ib, (i0, w) in enumerate(IBLOCKS):
                isl = slice(i0, i0 + w)
                num1 = nump.tile([D + 1, IBLK], F32, tag="num", name="num1")
                num2 = nump.tile([D + 1, IBLK], F32, tag="num", name="num2")
                prev_et = None
                for jt in range(NJT + 1):
                    if ib == 0 and jt % 2 == 1 and (jt + 3) // 2 < NCH:
                        emit_proj((jt + 3) // 2)
                    if jt < NJT:
                        j0 = jt * P
                        lt = ltp.tile([P, 2, IBLK], F32, tag="lt", name="lt")
                        nc.tensor.matmul(
                            lt[:, 0, :w], lhsT=g2x_sb[0:D, j0:j0 + P],
                            rhs=f12_sb[0:D, isl],
                            start=True, stop=True, tile_position=(0, 0))
                        nc.tensor.matmul(
                            lt[:, 1, :w], lhsT=g2x_sb[D:P, j0:j0 + P],
                            rhs=f12_sb[D:P, isl],
                            start=True, stop=True, tile_position=(64, 0))
                        et = ebuf.tile([P, 2, IBLK], F32R, tag="et", name="et")
                        nc.scalar.activation(
                            out=et[:, :, :w], in_=lt[:, :, :w],
                            func=AF.Exp, bias=shiftb[:, 0:1], scale=1.0)
                    if jt > 0:
                        pj = jt - 1
                        st, sp = (pj == 0), (pj == NJT - 1)
                        nc.tensor.matmul(
                            num1[:, :w], lhsT=hT_sb[:, pj, :],
                            rhs=prev_et[:, 0, :w], start=st, stop=sp)
                        nc.tensor.matmul(
                            num2[:, :w], lhsT=hT_sb[:, pj, :],
                            rhs=prev_et[:, 1, :w], start=st, stop=sp)
                    if jt < NJT:
                        prev_et = et
                    if jt == 3 and pending is not None:
                        emit_out(*pending)
                        pending = None
                rcp1 = rcpp.tile([1, IBLK], F32R, tag="rcp", name="rcp1")
                rcp2 = rcpp.tile([1, IBLK], F32R, tag="rcp", name="rcp2")
                with nc.allow_low_precision(
                        reason="softmax denominator reciprocal in f32r"):
                    nc.vector.reciprocal(rcp1[0:1, :w], num1[D:D + 1, :w])
                    nc.vector.reciprocal(rcp2[0:1, :w], num2[D:D + 1, :w])
                rb1 = scratch.tile([P, IBLK], F32, tag="sc", name="rb1")
                nc.tensor.matmul(
                    rb1[0:D, :w], lhsT=sel2[0:1, 0:D], rhs=rcp1[:, :w],
                    start=True, stop=True)
                rb2 = scratch.tile([P, IBLK], F32, tag="sc", name="rb2")
                nc.tensor.matmul(
                    rb2[0:D, :w], lhsT=sel2[0:1, 0:D], rhs=rcp2[:, :w],
                    start=True, stop=True)
                ea = eap.tile([P, IBLK], F32R, tag="ea", name="ea")
                nc.vector.tensor_copy(out=ea[0:D, :w], in_=num1[0:D, :w])
                nc.vector.tensor_copy(out=ea[D:P, :w], in_=num2[0:D, :w])
                nc.vector.tensor_mul(ea[0:D, :w], ea[0:D, :w], rb1[0:D, :w])
                nc.vector.tensor_mul(ea[D:P, :w], ea[D:P, :w], rb2[0:D, :w])
                pending = (ea, i0, w)
            emit_out(*pending)

    nc.compile()
    return nc


_NC_CACHE = None


def _get_nc():
    global _NC_CACHE
    if _NC_CACHE is None:
        _NC_CACHE = build_program()
    return _NC_CACHE


def _run(inputs, trace=False, trace_cores=None):
    from concourse.bass_utils import run_bass_kernel_spmd

    import ml_dtypes
    g = {k: np.ascontiguousarray(np.asarray(v, dtype=np.float32))
         for k, v in inputs.items()}
    x = g["x"].reshape(BS, C, N)
    y = g["y"].reshape(BS, C, N)
    x16 = np.ascontiguousarray(x.astype(ml_dtypes.bfloat16))
    y16 = np.ascontiguousarray(y.astype(ml_dtypes.bfloat16))

    def core_inputs(b, s):
        def sel(a0, a1):
            return a0 if s == 0 else a1

        gate1 = float(np.asarray(sel(g["alpha"], g["gamma"])).reshape(-1)[0])
        gate2 = float(np.asarray(sel(g["beta"], g["sigma"])).reshape(-1)[0])
        Wf1h = np.concatenate(
            [g["Wf1"].T, sel(g["Wh1"], g["Wh2"]).T], axis=1)   # [C, 128]
        WvS = np.concatenate(
            [gate1 * sel(g["Wv11"], g["Wv12"]).T,
             gate2 * sel(g["Wv21"], g["Wv22"]).T], axis=0)     # [128, C]
        cv = (gate1 * sel(g["bv11"], g["bv12"])
              + gate2 * sel(g["bv21"], g["bv22"]))             # [C]
        smalls = np.zeros((P, 8), np.float32)
        smalls[0:D, 0] = g["bf1"]
        smalls[D:P, 0] = sel(g["bh1"], g["bh2"])
        smalls[0:D, 1] = g["bf2"]
        smalls[0:D, 2] = sel(g["bg1"], g["bg2"])
        smalls[:, 4:8] = cv.reshape(NCT, P).T
        sel2 = np.zeros((2, P), np.float32)
        sel2[0, 0:D] = 1.0
        sel2[1, D:P] = 1.0
        return {
            "xin": np.ascontiguousarray(x16[b]),
            "yin": np.ascontiguousarray(y16[b]),
            "zin": np.ascontiguousarray(sel(x16, y16)[b]),
            "Wf1h": np.ascontiguousarray(Wf1h.astype(ml_dtypes.bfloat16)),
            "Wf2T": np.ascontiguousarray(
                g["Wf2"].T.astype(ml_dtypes.bfloat16)),
            "WgT": np.ascontiguousarray(
                sel(g["Wg1"], g["Wg2"]).T.astype(ml_dtypes.bfloat16)),
            "WvS": np.ascontiguousarray(WvS),
            "smalls": smalls,
            "sel": sel2,
        }

    in_maps = [core_inputs(core // 2, core % 2) for core in range(8)]
    res = run_bass_kernel_spmd(
        _get_nc(), in_maps, core_ids=list(range(8)), trace=trace,
        trace_cores=trace_cores)
    outs = [r["out"] for r in res.results]
    x_out = np.stack([outs[2 * b] for b in range(BS)]).reshape(BS, C, H, W)
    y_out = np.stack([outs[2 * b + 1] for b in range(BS)]).reshape(BS, C, H, W)
    return (x_out, y_out), res


def kernel(**inputs):
    out, _ = _run(inputs)
    return out


# revision 18
# speedup vs baseline: 1.1312x; 1.0151x over previous
"""Trainium2 Bass kernel for the Cross_Attention module.

Math (per batch b, per output stream):
  f1 = Wf1 @ x + bf1         [D, N]   (from x, both streams)
  f2 = Wf2 @ y + bf2         [D, N]   (from y, both streams)
  g  = Wg  @ z + bg          [D, N]   (z = x for the x_out stream, y for y_out)
  h  = Wh  @ x + bh          [D, N]   (always from x)
  A_a[i, j] = softmax_j(f_a[:, i] . g[:, j])        a in {1, 2}
  out = z + ga * (Wv1 @ (h A_1^T) + bv1) + gb * (Wv2 @ (h A_2^T) + bv2)

Sharding: 8 cores = 4 batches x 2 streams (x_out / y_out). No collectives.

Device algorithm (per core):
  - logits computed TRANSPOSED: LT[j, i] = sum_d g[d, j] f[d, i] so that the
    softmax reduction axis j lands on PSUM partitions. The two attentions
    share the stationary g tile and run as row-tiled bf16 matmuls
    (f1 in partitions 0:64, f2 in 64:128).
  - E = exp(LT - 40)  (constant shift; |logits| << 40+88 so exp is safe, and
    softmax is shift-invariant so the result is exact).
  - num[., i] = [hT | ones]^T @ E: one matmul per j-tile accumulates both the
    numerator (rows 0..63) and the softmax denominator (row 64).
  - EA = num[0:64] * (1/num[64]) broadcast via a K=2 selector matmul that
    serves both attentions at once.
  - out = z + [ga*Wv1 | gb*Wv2] @ [EA1; EA2] + (ga*bv1 + gb*bv2): the two
    value GEMMs are one K=128 matmul against host-stacked weights.
Projection chains are merged ([Wf1|Wh] is one K=128-wide stationary) and
pipelined in 256-column chunks against the input DMA stream, so compute
starts ~3us in and the attention loop is paced by the Activation engine
(the exp of 2*N^2 logits is the hard floor of this problem).
"""

import numpy as np

import concourse.bass as bass
import concourse.bacc as bacc
import concourse.mybir as mybir
import concourse.tile as tile
from concourse.masks import make_identity

BS = 4
C = 512
D = 64
H = W = 48
N = H * W          # 2304
P = 128
NK = C // P        # 4 contraction tiles for the projections
NCT = C // P       # 4 output channel tiles
NJT = N // P       # 18 j tiles
IBLK = 512
IBLOCKS = [(0, 512), (512, 512), (1024, 512), (1536, 512), (2048, 256)]
CHUNK = 256        # projection / input streaming chunk (columns)
NCH = N // CHUNK   # 9
SHIFT = 40.0

F32 = mybir.dt.float32
F32R = mybir.dt.float32r
BF16 = mybir.dt.bfloat16
AF = mybir.ActivationFunctionType
OP = mybir.AluOpType


def build_program():
    nc = bacc.Bacc("TRN2", target_bir_lowering=False)

    xin = nc.dram_tensor("xin", [C, N], BF16, kind="ExternalInput")
    yin = nc.dram_tensor("yin", [C, N], BF16, kind="ExternalInput")
    zin = nc.dram_tensor("zin", [C, N], BF16, kind="ExternalInput")
    # host-marshalled weights: Wf1h = [Wf1.T | Wh.T] (f1+h share one chain),
    # WvS = [ga*Wv1.T ; gb*Wv2.T] stacked on the contraction dim,
    # smalls = biases + cv packed: col0=[bf1;bh] col1=[bf2;-] col2=[bg;-]
    # cols 4:8 = cv = ga*bv1 + gb*bv2 in (ci, ct) layout.
    Wf1h = nc.dram_tensor("Wf1h", [C, P], BF16, kind="ExternalInput")
    Wf2T = nc.dram_tensor("Wf2T", [C, D], BF16, kind="ExternalInput")
    WgT = nc.dram_tensor("WgT", [C, D], BF16, kind="ExternalInput")
    WvS = nc.dram_tensor("WvS", [P, C], F32, kind="ExternalInput")
    smalls = nc.dram_tensor("smalls", [P, 8], F32, kind="ExternalInput")
    sel = nc.dram_tensor("sel", [2, P], F32, kind="ExternalInput")
    out = nc.dram_tensor("out", [C, N], F32, kind="ExternalOutput")

    xin_r = xin.rearrange("(co ci) n -> ci co n", ci=P)
    yin_r = yin.rearrange("(co ci) n -> ci co n", ci=P)
    zin_r = zin.rearrange("(co ci) n -> ci co n", ci=P)
    out_r = out.rearrange("(co ci) n -> ci co n", ci=P)

    with tile.TileContext(nc) as tc:
        with (
            tc.tile_pool(name="persist", bufs=1) as persist,
            tc.tile_pool(name="scratch", bufs=2, space="PSUM") as scratch,
            tc.tile_pool(name="ltp", bufs=2, space="PSUM") as ltp,
            tc.tile_pool(name="nump", bufs=2, space="PSUM") as nump,
            tc.tile_pool(name="ebuf", bufs=3) as ebuf,
            tc.tile_pool(name="eap", bufs=2) as eap,
            tc.tile_pool(name="rcpp", bufs=2) as rcpp,
            tc.tile_pool(name="osbp", bufs=2) as osbp,
        ):
            xin_sb = persist.tile([P, NK, N], BF16)
            yin_sb = persist.tile([P, NK, N], BF16)
            zin_sb = persist.tile([P, NK, N], BF16)

            # weights ride the gpsimd (SWDGE) queue so they land in parallel
            # with the input stream on the sync queue
            WgT_sb = persist.tile([P, NK, D], BF16)
            nc.gpsimd.dma_start(
                out=WgT_sb, in_=WgT.rearrange("(k ci) d -> ci k d", ci=P))
            Wf1h_sb = persist.tile([P, NK, P], BF16)
            nc.gpsimd.dma_start(
                out=Wf1h_sb, in_=Wf1h.rearrange("(k ci) d -> ci k d", ci=P))
            Wf2T_sb = persist.tile([P, NK, D], BF16)
            nc.gpsimd.dma_start(
                out=Wf2T_sb, in_=Wf2T.rearrange("(k ci) d -> ci k d", ci=P))
            smalls_sb = persist.tile([P, 8], F32)
            nc.gpsimd.dma_start(out=smalls_sb, in_=smalls[:, :])
            WvS_sb = persist.tile([P, NCT, P], F32R)
            nc.gpsimd.dma_start(
                out=WvS_sb,
                in_=WvS.rearrange("d (ct ci) -> d ct ci", ci=P).bitcast(F32R))

            # inputs stream in CHUNK-col slices; z first (g chain gates the
            # logits), then x (f1+h), then y (f2)
            for ch in range(NCH):
                sl = slice(ch * CHUNK, (ch + 1) * CHUNK)
                nc.sync.dma_start(out=zin_sb[:, :, sl], in_=zin_r[:, :, sl])
                nc.sync.dma_start(out=xin_sb[:, :, sl], in_=xin_r[:, :, sl])
                nc.sync.dma_start(out=yin_sb[:, :, sl], in_=yin_r[:, :, sl])

            # ---------------- constants ----------------
            # identity placed at partitions 64:128 (transposes read h from
            # the upper half of the f1h tile): ident2[x, y] = 1 iff x-64 == y
            ident2 = persist.tile([P, D], BF16)
            nc.gpsimd.memset(ident2, 0.0)
            nc.gpsimd.affine_select(
                out=ident2, in_=ident2,
                compare_op=mybir.AluOpType.not_equal, fill=1.0,
                base=-D, pattern=[[-1, D]], channel_multiplier=1)
            onesF = persist.tile([P, 1], F32)
            nc.vector.memset(onesF, 1.0)
            shiftb = persist.tile([P, 1], F32)
            nc.vector.memset(shiftb, -SHIFT)
            # dummy 1-element exp: pulls the ACT table load off the critical
            # path (runs during the input DMA head)
            dummy = persist.tile([1, 1], F32)
            nc.scalar.activation(
                out=dummy[0:1, 0:1], in_=shiftb[0:1, 0:1], func=AF.Exp,
                bias=shiftb[0:1, 0:1], scale=1.0)
            # selector for the K=2 reciprocal broadcast: row0 -> parts 0:64,
            # row1 -> parts 64:128 (host-supplied 0/1 matrix)
            sel2 = persist.tile([2, P], F32R)
            nc.gpsimd.dma_start(out=sel2, in_=sel[:, :].bitcast(F32R))

            # ---------------- persistent activations ----------------
            f1h_sb = persist.tile([P, N], BF16)   # rows 0:64 f1, 64:128 h
            f2_sb = persist.tile([D, N], BF16)
            g_sb = persist.tile([D, N], BF16)
            hT_sb = persist.tile([P, NJT, D + 1], BF16)
            nc.vector.tensor_copy(
                out=hT_sb[:, :, D],
                in_=onesF[:, 0:1].broadcast_to([P, NJT]))

            def emit_proj(ch):
                """Projection chains + hT transposes for a 256-col chunk."""
                i0 = ch * CHUNK
                w = CHUNK
                sl = slice(i0, i0 + w)
                pg = scratch.tile([P, IBLK], F32, tag="sc", name="pg")
                for k in range(NK):
                    nc.tensor.matmul(
                        pg[0:D, :w], lhsT=WgT_sb[:, k, :],
                        rhs=zin_sb[:, k, sl], start=(k == 0), stop=(k == NK - 1))
                nc.vector.tensor_scalar_add(
                    out=g_sb[:, sl], in0=pg[0:D, :w],
                    scalar1=smalls_sb[0:D, 2:3])
                pf = scratch.tile([P, IBLK], F32, tag="sc", name="pf")
                for k in range(NK):
                    nc.tensor.matmul(
                        pf[:, :w], lhsT=Wf1h_sb[:, k, :],
                        rhs=xin_sb[:, k, sl], start=(k == 0), stop=(k == NK - 1))
                nc.vector.tensor_scalar_add(
                    out=f1h_sb[:, sl], in0=pf[:, :w],
                    scalar1=smalls_sb[:, 0:1])
                pq = scratch.tile([P, IBLK], F32, tag="sc", name="pq")
                for k in range(NK):
                    nc.tensor.matmul(
                        pq[0:D, :w], lhsT=Wf2T_sb[:, k, :],
                        rhs=yin_sb[:, k, sl], start=(k == 0), stop=(k == NK - 1))
                nc.vector.tensor_scalar_add(
                    out=f2_sb[:, sl], in0=pq[0:D, :w],
                    scalar1=smalls_sb[0:D, 1:2])
                for jt in range(i0 // P, (i0 + w) // P):
                    pT = scratch.tile([P, IBLK], BF16, tag="sc", name="pT")
                    nc.tensor.transpose(
                        pT[:, 0:D], f1h_sb[D:P, jt * P:(jt + 1) * P],
                        ident2[D:P, 0:D])
                    nc.vector.tensor_copy(
                        out=hT_sb[:, jt, 0:D], in_=pT[:, 0:D])

            emit_proj(0)
            emit_proj(1)

            # ---------------- output projection ----------------
            def emit_out(ea, i0, w):
                osb = osbp.tile([P, NCT, IBLK], F32, tag="osb", name="osb")
                for ct in range(NCT):
                    # osb = z + cv first (waits only on the input stream),
                    # then += the value GEMM result
                    nc.vector.tensor_scalar_add(
                        out=osb[:, ct, :w], in0=zin_sb[:, ct, i0:i0 + w],
                        scalar1=smalls_sb[:, 4 + ct:5 + ct])
                    ops = scratch.tile([P, IBLK], F32, tag="sc", name="ops")
                    nc.tensor.matmul(
                        ops[:, :w], lhsT=WvS_sb[:, ct, :],
                        rhs=ea[:, :w], start=True, stop=True)
                    nc.vector.tensor_add(
                        osb[:, ct, :w], osb[:, ct, :w], ops[:, :w])
                nc.sync.dma_start(
                    out=out_r[:, :, i0:i0 + w], in_=osb[:, :, :w])

            # ---------------- attention main loop ----------------
            pending = None
            for ib, (i0, w) in enumerate(IBLOCKS):
                isl = slice(i0, i0 + w)
                num1 = nump.tile([D + 1, IBLK], F32, tag="num", name="num1")
                num2 = nump.tile([D + 1, IBLK], F32, tag="num", name="num2")
                prev_et = None
                for jt in range(NJT + 1):
                    if ib == 0 and jt % 2 == 1 and (jt + 3) // 2 < NCH:
                        emit_proj((jt + 3) // 2)
                    if jt < NJT:
                        j0 = jt * P
                        lt = ltp.tile([P, 2, IBLK], F32, tag="lt", name="lt")
                        nc.tensor.matmul(
                            lt[:, 0, :w], lhsT=g_sb[:, j0:j0 + P],
                            rhs=f1h_sb[0:D, isl], start=True, stop=True)
                        nc.tensor.matmul(
                            lt[:, 1, :w], lhsT=g_sb[:, j0:j0 + P],
                            rhs=f2_sb[:, isl], start=True, stop=True)
                        et = ebuf.tile([P, 2, IBLK], BF16, tag="et", name="et")
                        nc.scalar.activation(
                            out=et[:, :, :w], in_=lt[:, :, :w],
                            func=AF.Exp, bias=shiftb[:, 0:1], scale=1.0)
                    if jt > 0:
                        pj = jt - 1
                        st, sp = (pj == 0), (pj == NJT - 1)
                        nc.tensor.matmul(
                            num1[:, :w], lhsT=hT_sb[:, pj, :],
                            rhs=prev_et[:, 0, :w], start=st, stop=sp)
                        nc.tensor.matmul(
                            num2[:, :w], lhsT=hT_sb[:, pj, :],
                            rhs=prev_et[:, 1, :w], start=st, stop=sp)
                    if jt < NJT:
                        prev_et = et
                    if jt == 3 and pending is not None:
                        emit_out(*pending)
                        pending = None
                rcp1 = rcpp.tile([1, IBLK], F32R, tag="rcp", name="rcp1")
                rcp2 = rcpp.tile([1, IBLK], F32R, tag="rcp", name="rcp2")
                with nc.allow_low_precision(
                        reason="softmax denominator reciprocal in f32r"):
                    nc.vector.reciprocal(rcp1[0:1, :w], num1[D:D + 1, :w])
                    nc.vector.reciprocal(rcp2[0:1, :w], num2[D:D + 1, :w])
                rb1 = scratch.tile([P, IBLK], F32, tag="sc", name="rb1")
                nc.tensor.matmul(
                    rb1[0:D, :w], lhsT=sel2[0:1, 0:D], rhs=rcp1[:, :w],
                    start=True, stop=True)
                rb2 = scratch.tile([P, IBLK], F32, tag="sc", name="rb2")
                nc.tensor.matmul(
                    rb2[0:D, :w], lhsT=sel2[0:1, 0:D], rhs=rcp2[:, :w],
                    start=True, stop=True)
                ea = eap.tile([P, IBLK], F32R, tag="ea", name="ea")
                nc.vector.tensor_copy(out=ea[0:D, :w], in_=num1[0:D, :w])
                nc.vector.tensor_copy(out=ea[D:P, :w], in_=num2[0:D, :w])
                nc.vector.tensor_mul(ea[0:D, :w], ea[0:D, :w], rb1[0:D, :w])
                nc.vector.tensor_mul(ea[D:P, :w], ea[D:P, :w], rb2[0:D, :w])
                pending = (ea, i0, w)
            emit_out(*pending)

    nc.compile()
    return nc


_NC_CACHE = None


def _get_nc():
    global _NC_CACHE
    if _NC_CACHE is None:
        _NC_CACHE = build_program()
    return _NC_CACHE


def _run(inputs, trace=False, trace_cores=None):
    from concourse.bass_utils import run_bass_kernel_spmd

    import ml_dtypes
    g = {k: np.ascontiguousarray(np.asarray(v, dtype=np.float32))
         for k, v in inputs.items()}
    x = g["x"].reshape(BS, C, N)
    y = g["y"].reshape(BS, C, N)
    x16 = np.ascontiguousarray(x.astype(ml_dtypes.bfloat16))
    y16 = np.ascontiguousarray(y.astype(ml_dtypes.bfloat16))

    def core_inputs(b, s):
        def sel(a0, a1):
            return a0 if s == 0 else a1

        gate1 = float(np.asarray(sel(g["alpha"], g["gamma"])).reshape(-1)[0])
        gate2 = float(np.asarray(sel(g["beta"], g["sigma"])).reshape(-1)[0])
        Wf1h = np.concatenate(
            [g["Wf1"].T, sel(g["Wh1"], g["Wh2"]).T], axis=1)   # [C, 128]
        WvS = np.concatenate(
            [gate1 * sel(g["Wv11"], g["Wv12"]).T,
             gate2 * sel(g["Wv21"], g["Wv22"]).T], axis=0)     # [128, C]
        cv = (gate1 * sel(g["bv11"], g["bv12"])
              + gate2 * sel(g["bv21"], g["bv22"]))             # [C]
        smalls = np.zeros((P, 8), np.float32)
        smalls[0:D, 0] = g["bf1"]
        smalls[D:P, 0] = sel(g["bh1"], g["bh2"])
        smalls[0:D, 1] = g["bf2"]
        smalls[0:D, 2] = sel(g["bg1"], g["bg2"])
        smalls[:, 4:8] = cv.reshape(NCT, P).T
        sel2 = np.zeros((2, P), np.float32)
        sel2[0, 0:D] = 1.0
        sel2[1, D:P] = 1.0
        return {
            "xin": np.ascontiguousarray(x16[b]),
            "yin": np.ascontiguousarray(y16[b]),
            "zin": np.ascontiguousarray(sel(x16, y16)[b]),
            "Wf1h": np.ascontiguousarray(Wf1h.astype(ml_dtypes.bfloat16)),
            "Wf2T": np.ascontiguousarray(
                g["Wf2"].T.astype(ml_dtypes.bfloat16)),
            "WgT": np.ascontiguousarray(
                sel(g["Wg1"], g["Wg2"]).T.astype(ml_dtypes.bfloat16)),
            "WvS": np.ascontiguousarray(WvS),
            "smalls": smalls,
            "sel": sel2,
        }

    in_maps = [core_inputs(core // 2, core % 2) for core in range(8)]
    res = run_bass_kernel_spmd(
        _get_nc(), in_maps, core_ids=list(range(8)), trace=trace,
        trace_cores=trace_cores)
    outs = [r["out"] for r in res.results]
    x_out = np.stack([outs[2 * b] for b in range(BS)]).reshape(BS, C, H, W)
    y_out = np.stack([outs[2 * b + 1] for b in range(BS)]).reshape(BS, C, H, W)
    return (x_out, y_out), res


def kernel(**inputs):
    out, _ = _run(inputs)
    return out


# revision 23
# speedup vs baseline: 1.1539x; 1.0201x over previous
"""Trainium2 Bass kernel for the Cross_Attention module.

Math (per batch b, per output stream):
  f1 = Wf1 @ x + bf1         [D, N]   (from x, both streams)
  f2 = Wf2 @ y + bf2         [D, N]   (from y, both streams)
  g  = Wg  @ z + bg          [D, N]   (z = x for the x_out stream, y for y_out)
  h  = Wh  @ x + bh          [D, N]   (always from x)
  A_a[i, j] = softmax_j(f_a[:, i] . g[:, j])        a in {1, 2}
  out = z + ga * (Wv1 @ (h A_1^T) + bv1) + gb * (Wv2 @ (h A_2^T) + bv2)

Sharding: 8 cores = 4 batches x 2 streams (x_out / y_out). No collectives.

Device algorithm (per core):
  - logits computed TRANSPOSED: LT[j, i] = sum_d g[d, j] f[d, i] so that the
    softmax reduction axis j lands on PSUM partitions. The two attentions
    share the stationary g tile and run as row-tiled bf16 matmuls
    (f1 in partitions 0:64, f2 in 64:128).
  - E = exp(LT - 40)  (constant shift; |logits| << 40+88 so exp is safe, and
    softmax is shift-invariant so the result is exact).
  - num[., i] = [hT | ones]^T @ E: one matmul per j-tile accumulates both the
    numerator (rows 0..63) and the softmax denominator (row 64).
  - EA = num[0:64] * (1/num[64]) broadcast via a K=2 selector matmul that
    serves both attentions at once.
  - out = z + [ga*Wv1 | gb*Wv2] @ [EA1; EA2] + (ga*bv1 + gb*bv2): the two
    value GEMMs are one K=128 matmul against host-stacked weights.
Projection chains are merged ([Wf1|Wh] is one K=128-wide stationary) and
pipelined in 256-column chunks against the input DMA stream, so compute
starts ~3us in and the attention loop is paced by the Activation engine
(the exp of 2*N^2 logits is the hard floor of this problem).
"""

import numpy as np

import concourse.bass as bass
import concourse.bacc as bacc
import concourse.mybir as mybir
import concourse.tile as tile
from concourse.masks import make_identity

BS = 4
C = 512
D = 64
H = W = 48
N = H * W          # 2304
P = 128
NK = C // P        # 4 contraction tiles for the projections
NCT = C // P       # 4 output channel tiles
NJT = N // P       # 18 j tiles
IBLK = 512
IBLOCKS = [(0, 512), (512, 512), (1024, 512), (1536, 512), (2048, 256)]
CHUNK = 256        # projection / input streaming chunk (columns)
NCH = N // CHUNK   # 9
SHIFT = 40.0

F32 = mybir.dt.float32
F32R = mybir.dt.float32r
BF16 = mybir.dt.bfloat16
AF = mybir.ActivationFunctionType
OP = mybir.AluOpType


def build_program():
    nc = bacc.Bacc("TRN2", target_bir_lowering=False)

    xin = nc.dram_tensor("xin", [C, N], BF16, kind="ExternalInput")
    yin = nc.dram_tensor("yin", [C, N], BF16, kind="ExternalInput")
    zin = nc.dram_tensor("zin", [C, N], BF16, kind="ExternalInput")
    # host-marshalled weights: Wf1h = [Wf1.T | Wh.T] (f1+h share one chain),
    # WvS = [ga*Wv1.T ; gb*Wv2.T] stacked on the contraction dim,
    # smalls = biases + cv packed: col0=[bf1;bh] col1=[bf2;-] col2=[bg;-]
    # cols 4:8 = cv = ga*bv1 + gb*bv2 in (ci, ct) layout.
    Wf1h = nc.dram_tensor("Wf1h", [C, P], BF16, kind="ExternalInput")
    Wf2T = nc.dram_tensor("Wf2T", [C, D], BF16, kind="ExternalInput")
    WgT = nc.dram_tensor("WgT", [C, D], BF16, kind="ExternalInput")
    WvS = nc.dram_tensor("WvS", [P, C], F32, kind="ExternalInput")
    smalls = nc.dram_tensor("smalls", [P, 8], F32, kind="ExternalInput")
    sel = nc.dram_tensor("sel", [2, P], F32, kind="ExternalInput")
    out = nc.dram_tensor("out", [C, N], F32, kind="ExternalOutput")

    xin_r = xin.rearrange("(co ci) n -> ci co n", ci=P)
    yin_r = yin.rearrange("(co ci) n -> ci co n", ci=P)
    zin_r = zin.rearrange("(co ci) n -> ci co n", ci=P)
    out_r = out.rearrange("(co ci) n -> ci co n", ci=P)

    with tile.TileContext(nc) as tc:
        with (
            tc.tile_pool(name="persist", bufs=1) as persist,
            tc.tile_pool(name="scratch", bufs=2, space="PSUM") as scratch,
            tc.tile_pool(name="ltp", bufs=2, space="PSUM") as ltp,
            tc.tile_pool(name="nump", bufs=2, space="PSUM") as nump,
            tc.tile_pool(name="ebuf", bufs=3) as ebuf,
            tc.tile_pool(name="eap", bufs=2) as eap,
            tc.tile_pool(name="rcpp", bufs=2) as rcpp,
            tc.tile_pool(name="osbp", bufs=2) as osbp,
        ):
            xin_sb = persist.tile([P, NK, N], BF16)
            yin_sb = persist.tile([P, NK, N], BF16)
            zin_sb = persist.tile([P, NK, N], BF16)

            # projection weights + biases go FIRST on the sync queue: small
            # (0.45 MB) but they gate the first projection chains; the input
            # chunks stream right behind them
            smalls_sb = persist.tile([P, 8], F32)
            nc.sync.dma_start(out=smalls_sb, in_=smalls[:, :])
            WgT_sb = persist.tile([P, NK, D], BF16)
            nc.sync.dma_start(
                out=WgT_sb, in_=WgT.rearrange("(k ci) d -> ci k d", ci=P))
            Wf1h_sb = persist.tile([P, NK, P], BF16)
            nc.sync.dma_start(
                out=Wf1h_sb, in_=Wf1h.rearrange("(k ci) d -> ci k d", ci=P))
            Wf2T_sb = persist.tile([P, NK, D], BF16)
            nc.sync.dma_start(
                out=Wf2T_sb, in_=Wf2T.rearrange("(k ci) d -> ci k d", ci=P))
            # value weights + selector are needed only ~25us in; they ride
            # the gpsimd (SWDGE) queue
            WvS_sb = persist.tile([P, NCT, P], F32R)
            nc.gpsimd.dma_start(
                out=WvS_sb,
                in_=WvS.rearrange("d (ct ci) -> d ct ci", ci=P).bitcast(F32R))

            # inputs stream in CHUNK-col slices; z first (g chain gates the
            # logits), then x (f1+h), then y (f2)
            for ch in range(NCH):
                sl = slice(ch * CHUNK, (ch + 1) * CHUNK)
                nc.sync.dma_start(out=zin_sb[:, :, sl], in_=zin_r[:, :, sl])
                nc.sync.dma_start(out=xin_sb[:, :, sl], in_=xin_r[:, :, sl])
                nc.sync.dma_start(out=yin_sb[:, :, sl], in_=yin_r[:, :, sl])

            # ---------------- constants ----------------
            # identity placed at partitions 64:128 (transposes read h from
            # the upper half of the f1h tile): ident2[x, y] = 1 iff x-64 == y
            ident2 = persist.tile([P, D], BF16)
            nc.gpsimd.memset(ident2, 0.0)
            nc.gpsimd.affine_select(
                out=ident2, in_=ident2,
                compare_op=mybir.AluOpType.not_equal, fill=1.0,
                base=-D, pattern=[[-1, D]], channel_multiplier=1)
            onesF = persist.tile([P, 1], F32)
            nc.vector.memset(onesF, 1.0)
            shiftb = persist.tile([P, 1], F32)
            nc.vector.memset(shiftb, -SHIFT)
            # dummy 1-element exp: pulls the ACT table load off the critical
            # path (runs during the input DMA head)
            dummy = persist.tile([1, 1], F32)
            nc.scalar.activation(
                out=dummy[0:1, 0:1], in_=shiftb[0:1, 0:1], func=AF.Exp,
                bias=shiftb[0:1, 0:1], scale=1.0)
            # selector for the K=2 reciprocal broadcast: row0 -> parts 0:64,
            # row1 -> parts 64:128 (host-supplied 0/1 matrix)
            sel2 = persist.tile([2, P], F32R)
            nc.gpsimd.dma_start(out=sel2, in_=sel[:, :].bitcast(F32R))

            # ---------------- persistent activations ----------------
            f1h_sb = persist.tile([P, N], BF16)   # rows 0:64 f1, 64:128 h
            f2_sb = persist.tile([D, N], BF16)
            g_sb = persist.tile([D, N], BF16)
            hT_sb = persist.tile([P, NJT, D + 1], BF16)
            nc.vector.tensor_copy(
                out=hT_sb[:, :, D],
                in_=onesF[:, 0:1].broadcast_to([P, NJT]))

            def emit_proj(ch):
                """Projection chains + hT transposes for a 256-col chunk."""
                i0 = ch * CHUNK
                w = CHUNK
                sl = slice(i0, i0 + w)
                pg = scratch.tile([P, IBLK], F32, tag="sc", name="pg")
                for k in range(NK):
                    nc.tensor.matmul(
                        pg[0:D, :w], lhsT=WgT_sb[:, k, :],
                        rhs=zin_sb[:, k, sl], start=(k == 0), stop=(k == NK - 1))
                nc.vector.tensor_scalar_add(
                    out=g_sb[:, sl], in0=pg[0:D, :w],
                    scalar1=smalls_sb[0:D, 2:3])
                pf = scratch.tile([P, IBLK], F32, tag="sc", name="pf")
                for k in range(NK):
                    nc.tensor.matmul(
                        pf[:, :w], lhsT=Wf1h_sb[:, k, :],
                        rhs=xin_sb[:, k, sl], start=(k == 0), stop=(k == NK - 1))
                nc.vector.tensor_scalar_add(
                    out=f1h_sb[:, sl], in0=pf[:, :w],
                    scalar1=smalls_sb[:, 0:1])
                pq = scratch.tile([P, IBLK], F32, tag="sc", name="pq")
                for k in range(NK):
                    nc.tensor.matmul(
                        pq[0:D, :w], lhsT=Wf2T_sb[:, k, :],
                        rhs=yin_sb[:, k, sl], start=(k == 0), stop=(k == NK - 1))
                nc.vector.tensor_scalar_add(
                    out=f2_sb[:, sl], in0=pq[0:D, :w],
                    scalar1=smalls_sb[0:D, 1:2])

            def emit_transposes(ch):
                i0 = ch * CHUNK
                for jt in range(i0 // P, (i0 + CHUNK) // P):
                    pT = scratch.tile([P, IBLK], BF16, tag="sc", name="pT")
                    nc.tensor.transpose(
                        pT[:, 0:D], f1h_sb[D:P, jt * P:(jt + 1) * P],
                        ident2[D:P, 0:D])
                    nc.vector.tensor_copy(
                        out=hT_sb[:, jt, 0:D], in_=pT[:, 0:D])

            emit_proj(0)
            emit_proj(1)

            # ---------------- output projection ----------------
            # one channel-tile per call so the work spreads across j-loop
            # iterations instead of stalling PE in one burst
            def emit_out_start(ea, i0, w):
                return {"ea": ea, "i0": i0, "w": w,
                        "osb": osbp.tile([P, NCT, IBLK], F32,
                                         tag="osb", name="osb")}

            def emit_out_ct(st, ct):
                ea, i0, w, osb = st["ea"], st["i0"], st["w"], st["osb"]
                # osb = z + cv first (waits only on the input stream),
                # then += the value GEMM result
                nc.vector.tensor_scalar_add(
                    out=osb[:, ct, :w], in0=zin_sb[:, ct, i0:i0 + w],
                    scalar1=smalls_sb[:, 4 + ct:5 + ct])
                ops = scratch.tile([P, IBLK], F32, tag="sc", name="ops")
                nc.tensor.matmul(
                    ops[:, :w], lhsT=WvS_sb[:, ct, :],
                    rhs=ea[:, :w], start=True, stop=True)
                nc.vector.tensor_add(
                    osb[:, ct, :w], osb[:, ct, :w], ops[:, :w])
                if ct == NCT - 1:
                    nc.sync.dma_start(
                        out=out_r[:, :, i0:i0 + w], in_=osb[:, :, :w])

            # ---------------- attention main loop ----------------
            pending = None
            for ib, (i0, w) in enumerate(IBLOCKS):
                isl = slice(i0, i0 + w)
                num1 = nump.tile([D + 1, IBLK], F32, tag="num", name="num1")
                num2 = nump.tile([D + 1, IBLK], F32, tag="num", name="num2")
                prev_et = None
                for jt in range(NJT + 1):
                    if ib == 0 and jt % 2 == 1 and (jt + 3) // 2 < NCH:
                        emit_proj((jt + 3) // 2)
                        emit_transposes((jt + 3) // 2)
                    if jt < NJT:
                        j0 = jt * P
                        lt = ltp.tile([P, 2, IBLK], F32, tag="lt", name="lt")
                        nc.tensor.matmul(
                            lt[:, 0, :w], lhsT=g_sb[:, j0:j0 + P],
                            rhs=f1h_sb[0:D, isl], start=True, stop=True)
                        nc.tensor.matmul(
                            lt[:, 1, :w], lhsT=g_sb[:, j0:j0 + P],
                            rhs=f2_sb[:, isl], start=True, stop=True)
                        et = ebuf.tile([P, 2, IBLK], BF16, tag="et", name="et")
                        nc.scalar.activation(
                            out=et[:, :, :w], in_=lt[:, :, :w],
                            func=AF.Exp, bias=shiftb[:, 0:1], scale=1.0)
                    if ib == 0 and jt == 0:
                        emit_transposes(0)
                        emit_transposes(1)
                    if jt > 0:
                        pj = jt - 1
                        st, sp = (pj == 0), (pj == NJT - 1)
                        nc.tensor.matmul(
                            num1[:, :w], lhsT=hT_sb[:, pj, :],
                            rhs=prev_et[:, 0, :w], start=st, stop=sp)
                        nc.tensor.matmul(
                            num2[:, :w], lhsT=hT_sb[:, pj, :],
                            rhs=prev_et[:, 1, :w], start=st, stop=sp)
                    if jt < NJT:
                        prev_et = et
                    if pending is not None and jt in (6, 8, 10, 12):
                        emit_out_ct(pending, (jt - 6) // 2)
                        if jt == 12:
                            pending = None
                rcp1 = rcpp.tile([1, IBLK], F32R, tag="rcp", name="rcp1")
                rcp2 = rcpp.tile([1, IBLK], F32R, tag="rcp", name="rcp2")
                with nc.allow_low_precision(
                        reason="softmax denominator reciprocal in f32r"):
                    nc.vector.reciprocal(rcp1[0:1, :w], num1[D:D + 1, :w])
                    nc.vector.reciprocal(rcp2[0:1, :w], num2[D:D + 1, :w])
                rb1 = scratch.tile([P, IBLK], F32, tag="sc", name="rb1")
                nc.tensor.matmul(
                    rb1[0:D, :w], lhsT=sel2[0:1, 0:D], rhs=rcp1[:, :w],
                    start=True, stop=True)
                rb2 = scratch.tile([P, IBLK], F32, tag="sc", name="rb2")
                nc.tensor.matmul(
                    rb2[0:D, :w], lhsT=sel2[0:1, 0:D], rhs=rcp2[:, :w],
                    start=True, stop=True)
                ea = eap.tile([P, IBLK], F32R, tag="ea", name="ea")
                nc.vector.tensor_copy(out=ea[0:D, :w], in_=num1[0:D, :w])
                nc.vector.tensor_copy(out=ea[D:P, :w], in_=num2[0:D, :w])
                nc.vector.tensor_mul(ea[0:D, :w], ea[0:D, :w], rb1[0:D, :w])
                nc.vector.tensor_mul(ea[D:P, :w], ea[D:P, :w], rb2[0:D, :w])
                pending = emit_out_start(ea, i0, w)
            for ct in range(NCT):
                emit_out_ct(pending, ct)

    nc.compile()
    return nc


_NC_CACHE = None


def _get_nc():
    global _NC_CACHE
    if _NC_CACHE is None:
        _NC_CACHE = build_program()
    return _NC_CACHE


def _run(inputs, trace=False, trace_cores=None):
    from concourse.bass_utils import run_bass_kernel_spmd

    import ml_dtypes
    g = {k: np.ascontiguousarray(np.asarray(v, dtype=np.float32))
         for k, v in inputs.items()}
    x = g["x"].reshape(BS, C, N)
    y = g["y"].reshape(BS, C, N)
    x16 = np.ascontiguousarray(x.astype(ml_dtypes.bfloat16))
    y16 = np.ascontiguousarray(y.astype(ml_dtypes.bfloat16))

    def core_inputs(b, s):
        def sel(a0, a1):
            return a0 if s == 0 else a1

        gate1 = float(np.asarray(sel(g["alpha"], g["gamma"])).reshape(-1)[0])
        gate2 = float(np.asarray(sel(g["beta"], g["sigma"])).reshape(-1)[0])
        Wf1h = np.concatenate(
            [g["Wf1"].T, sel(g["Wh1"], g["Wh2"]).T], axis=1)   # [C, 128]
        WvS = np.concatenate(
            [gate1 * sel(g["Wv11"], g["Wv12"]).T,
             gate2 * sel(g["Wv21"], g["Wv22"]).T], axis=0)     # [128, C]
        cv = (gate1 * sel(g["bv11"], g["bv12"])
              + gate2 * sel(g["bv21"], g["bv22"]))             # [C]
        smalls = np.zeros((P, 8), np.float32)
        smalls[0:D, 0] = g["bf1"]
        smalls[D:P, 0] = sel(g["bh1"], g["bh2"])
        smalls[0:D, 1] = g["bf2"]
        smalls[0:D, 2] = sel(g["bg1"], g["bg2"])
        smalls[:, 4:8] = cv.reshape(NCT, P).T
        sel2 = np.zeros((2, P), np.float32)
        sel2[0, 0:D] = 1.0
        sel2[1, D:P] = 1.0
        return {
            "xin": np.ascontiguousarray(x16[b]),
            "yin": np.ascontiguousarray(y16[b]),
            "zin": np.ascontiguousarray(sel(x16, y16)[b]),
            "Wf1h": np.ascontiguousarray(Wf1h.astype(ml_dtypes.bfloat16)),
            "Wf2T": np.ascontiguousarray(
                g["Wf2"].T.astype(ml_dtypes.bfloat16)),
            "WgT": np.ascontiguousarray(
                sel(g["Wg1"], g["Wg2"]).T.astype(ml_dtypes.bfloat16)),
            "WvS": np.ascontiguousarray(WvS),
            "smalls": smalls,
            "sel": sel2,
        }

    in_maps = [core_inputs(core // 2, core % 2) for core in range(8)]
    res = run_bass_kernel_spmd(
        _get_nc(), in_maps, core_ids=list(range(8)), trace=trace,
        trace_cores=trace_cores)
    outs = [r["out"] for r in res.results]
    x_out = np.stack([outs[2 * b] for b in range(BS)]).reshape(BS, C, H, W)
    y_out = np.stack([outs[2 * b + 1] for b in range(BS)]).reshape(BS, C, H, W)
    return (x_out, y_out), res


def kernel(**inputs):
    out, _ = _run(inputs)
    return out


# revision 30
# speedup vs baseline: 1.2079x; 1.0468x over previous
"""Trainium2 Bass kernel for the Cross_Attention module.

Math (per batch b, per output stream):
  f1 = Wf1 @ x + bf1         [D, N]   (from x, both streams)
  f2 = Wf2 @ y + bf2         [D, N]   (from y, both streams)
  g  = Wg  @ z + bg          [D, N]   (z = x for the x_out stream, y for y_out)
  h  = Wh  @ x + bh          [D, N]   (always from x)
  A_a[i, j] = softmax_j(f_a[:, i] . g[:, j])        a in {1, 2}
  out = z + ga * (Wv1 @ (h A_1^T) + bv1) + gb * (Wv2 @ (h A_2^T) + bv2)

Sharding: 8 cores = 4 batches x 2 streams (x_out / y_out). No collectives.

Device algorithm (per core):
  - logits computed TRANSPOSED: LT[j, i] = sum_d g[d, j] f[d, i] so that the
    softmax reduction axis j lands on PSUM partitions. The two attentions
    share the stationary g tile and run as row-tiled bf16 matmuls
    (f1 in partitions 0:64, f2 in 64:128).
  - E = exp(LT - 40)  (constant shift; |logits| << 40+88 so exp is safe, and
    softmax is shift-invariant so the result is exact).
  - num[., i] = [hT | ones]^T @ E: one matmul per j-tile accumulates both the
    numerator (rows 0..63) and the softmax denominator (row 64).
  - EA = num[0:64] * (1/num[64]) broadcast via a K=2 selector matmul that
    serves both attentions at once.
  - out = z + [ga*Wv1 | gb*Wv2] @ [EA1; EA2] + (ga*bv1 + gb*bv2): the two
    value GEMMs are one K=128 matmul against host-stacked weights.
Projection chains are merged ([Wf1|Wh] is one K=128-wide stationary) and
pipelined in 256-column chunks against the input DMA stream, so compute
starts ~3us in and the attention loop is paced by the Activation engine
(the exp of 2*N^2 logits is the hard floor of this problem).
"""

import numpy as np

import concourse.bass as bass
import concourse.bacc as bacc
import concourse.mybir as mybir
import concourse.tile as tile
from concourse.masks import make_identity

BS = 4
C = 512
D = 64
H = W = 48
N = H * W          # 2304
P = 128
NK = C // P        # 4 contraction tiles for the projections
NCT = C // P       # 4 output channel tiles
NJT = N // P       # 18 j tiles
IBLK = 512
IBLOCKS = [(0, 512), (512, 512), (1024, 512), (1536, 512), (2048, 256)]
CHUNK = 256        # projection / input streaming chunk (columns)
NCH = N // CHUNK   # 9
SHIFT = 40.0

F32 = mybir.dt.float32
F32R = mybir.dt.float32r
BF16 = mybir.dt.bfloat16
AF = mybir.ActivationFunctionType
OP = mybir.AluOpType


def build_program():
    nc = bacc.Bacc("TRN2", target_bir_lowering=False)

    xin = nc.dram_tensor("xin", [C, N], BF16, kind="ExternalInput")
    yin = nc.dram_tensor("yin", [C, N], BF16, kind="ExternalInput")
    zin = nc.dram_tensor("zin", [C, N], BF16, kind="ExternalInput")
    # host-marshalled weights: Wf1h = [Wf1.T | Wh.T] (f1+h share one chain),
    # WvS = [ga*Wv1.T ; gb*Wv2.T] stacked on the contraction dim,
    # smalls = biases + cv packed: col0=[bf1;bh] col1=[bf2;-] col2=[bg;-]
    # cols 4:8 = cv = ga*bv1 + gb*bv2 in (ci, ct) layout.
    # Wcat = [WgT | Wf1T | WhT | Wf2T] in one DMA-friendly block
    Wcat = nc.dram_tensor("Wcat", [C, 4 * D], BF16, kind="ExternalInput")
    WvS = nc.dram_tensor("WvS", [P, C], F32, kind="ExternalInput")
    smalls = nc.dram_tensor("smalls", [P, 8], F32, kind="ExternalInput")
    sel = nc.dram_tensor("sel", [2, P], F32, kind="ExternalInput")
    out = nc.dram_tensor("out", [C, N], F32, kind="ExternalOutput")

    xin_r = xin.rearrange("(co ci) n -> ci co n", ci=P)
    yin_r = yin.rearrange("(co ci) n -> ci co n", ci=P)
    zin_r = zin.rearrange("(co ci) n -> ci co n", ci=P)
    out_r = out.rearrange("(co ci) n -> ci co n", ci=P)

    with tile.TileContext(nc) as tc:
        with (
            tc.tile_pool(name="persist", bufs=1) as persist,
            tc.tile_pool(name="scratch", bufs=2, space="PSUM") as scratch,
            tc.tile_pool(name="ltp", bufs=2, space="PSUM") as ltp,
            tc.tile_pool(name="nump", bufs=2, space="PSUM") as nump,
            tc.tile_pool(name="ebuf", bufs=3) as ebuf,
            tc.tile_pool(name="eap", bufs=2) as eap,
            tc.tile_pool(name="rcpp", bufs=2) as rcpp,
            tc.tile_pool(name="osbp", bufs=3) as osbp,
        ):
            xin_sb = persist.tile([P, NK, N], BF16)
            yin_sb = persist.tile([P, NK, N], BF16)
            zin_sb = persist.tile([P, NK, N], BF16)

            # projection weights + biases go FIRST on the sync queue: small
            # (0.45 MB) but they gate the first projection chains; the input
            # chunks stream right behind them
            smalls_sb = persist.tile([P, 8], F32)
            nc.sync.dma_start(out=smalls_sb, in_=smalls[:, :])
            Wcat_sb = persist.tile([P, NK, 4 * D], BF16)
            nc.sync.dma_start(
                out=Wcat_sb, in_=Wcat.rearrange("(k ci) d -> ci k d", ci=P))
            WgT_sb = Wcat_sb[:, :, 0:D]
            Wf1h_sb = Wcat_sb[:, :, D:3 * D]
            Wf2T_sb = Wcat_sb[:, :, 3 * D:4 * D]
            # value weights + selector are needed only ~25us in; they ride
            # the gpsimd (SWDGE) queue
            WvS_sb = persist.tile([P, NCT, P], F32R)
            nc.gpsimd.dma_start(
                out=WvS_sb,
                in_=WvS.rearrange("d (ct ci) -> d ct ci", ci=P).bitcast(F32R))

            # inputs stream in CHUNK-col slices; z first (g chain gates the
            # logits), then x (f1+h), then y (f2)
            for ch in range(NCH):
                sl = slice(ch * CHUNK, (ch + 1) * CHUNK)
                nc.sync.dma_start(out=zin_sb[:, :, sl], in_=zin_r[:, :, sl])
                nc.sync.dma_start(out=xin_sb[:, :, sl], in_=xin_r[:, :, sl])
                nc.sync.dma_start(out=yin_sb[:, :, sl], in_=yin_r[:, :, sl])

            # ---------------- constants ----------------
            # identity placed at partitions 64:128 (transposes read h from
            # the upper half of the f1h tile): ident2[x, y] = 1 iff x-64 == y
            ident2 = persist.tile([P, D], BF16)
            nc.gpsimd.memset(ident2, 0.0)
            nc.gpsimd.affine_select(
                out=ident2, in_=ident2,
                compare_op=mybir.AluOpType.not_equal, fill=1.0,
                base=-D, pattern=[[-1, D]], channel_multiplier=1)
            onesF = persist.tile([P, 1], F32)
            nc.vector.memset(onesF, 1.0)
            shiftb = persist.tile([P, 1], F32)
            nc.vector.memset(shiftb, -SHIFT)
            # dummy 1-element exp: pulls the ACT table load off the critical
            # path (runs during the input DMA head)
            dummy = persist.tile([1, 1], F32)
            nc.scalar.activation(
                out=dummy[0:1, 0:1], in_=shiftb[0:1, 0:1], func=AF.Exp,
                bias=shiftb[0:1, 0:1], scale=1.0)
            # selector for the K=2 reciprocal broadcast: row0 -> parts 0:64,
            # row1 -> parts 64:128 (host-supplied 0/1 matrix)
            sel2 = persist.tile([2, P], F32R)
            nc.gpsimd.dma_start(out=sel2, in_=sel[:, :].bitcast(F32R))

            # ---------------- persistent activations ----------------
            f1h_sb = persist.tile([P, N], BF16)   # rows 0:64 f1, 64:128 h
            f2_sb = persist.tile([D, N], BF16)
            g_sb = persist.tile([D, N], BF16)
            hT_sb = persist.tile([P, NJT, D + 1], BF16)
            nc.vector.tensor_copy(
                out=hT_sb[:, :, D],
                in_=onesF[:, 0:1].broadcast_to([P, NJT]))

            def emit_proj(ch):
                """Projection chains + hT transposes for a 256-col chunk."""
                i0 = ch * CHUNK
                w = CHUNK
                sl = slice(i0, i0 + w)
                pg = scratch.tile([P, IBLK], F32, tag="sc", name="pg")
                for k in range(NK):
                    nc.tensor.matmul(
                        pg[0:D, :w], lhsT=WgT_sb[:, k, :],
                        rhs=zin_sb[:, k, sl], start=(k == 0), stop=(k == NK - 1))
                nc.vector.tensor_scalar_add(
                    out=g_sb[:, sl], in0=pg[0:D, :w],
                    scalar1=smalls_sb[0:D, 2:3])
                pf = scratch.tile([P, IBLK], F32, tag="sc", name="pf")
                for k in range(NK):
                    nc.tensor.matmul(
                        pf[:, :w], lhsT=Wf1h_sb[:, k, :],
                        rhs=xin_sb[:, k, sl], start=(k == 0), stop=(k == NK - 1))
                nc.vector.tensor_scalar_add(
                    out=f1h_sb[:, sl], in0=pf[:, :w],
                    scalar1=smalls_sb[:, 0:1])
                pq = scratch.tile([P, IBLK], F32, tag="sc", name="pq")
                for k in range(NK):
                    nc.tensor.matmul(
                        pq[0:D, :w], lhsT=Wf2T_sb[:, k, :],
                        rhs=yin_sb[:, k, sl], start=(k == 0), stop=(k == NK - 1))
                nc.vector.tensor_scalar_add(
                    out=f2_sb[:, sl], in0=pq[0:D, :w],
                    scalar1=smalls_sb[0:D, 1:2])

            def emit_transposes(ch):
                i0 = ch * CHUNK
                for jt in range(i0 // P, (i0 + CHUNK) // P):
                    pT = scratch.tile([P, IBLK], BF16, tag="sc", name="pT")
                    nc.tensor.transpose(
                        pT[:, 0:D], f1h_sb[D:P, jt * P:(jt + 1) * P],
                        ident2[D:P, 0:D])
                    nc.vector.tensor_copy(
                        out=hT_sb[:, jt, 0:D], in_=pT[:, 0:D])

            emit_proj(0)
            emit_proj(1)

            # ---------------- output projection ----------------
            # one channel-tile per call so the work spreads across j-loop
            # iterations instead of stalling PE in one burst
            def emit_out_start(ea, i0, w):
                return {"ea": ea, "i0": i0, "w": w}

            def emit_out_ct(st, ct):
                ea, i0, w = st["ea"], st["i0"], st["w"]
                osb = osbp.tile([P, IBLK], F32, tag="osb", name="osb")
                # osb = z + cv first (waits only on the input stream),
                # then += the value GEMM result
                nc.vector.tensor_scalar_add(
                    out=osb[:, :w], in0=zin_sb[:, ct, i0:i0 + w],
                    scalar1=smalls_sb[:, 4 + ct:5 + ct])
                ops = scratch.tile([P, IBLK], F32, tag="sc", name="ops")
                nc.tensor.matmul(
                    ops[:, :w], lhsT=WvS_sb[:, ct, :],
                    rhs=ea[:, :w], start=True, stop=True)
                nc.vector.tensor_add(
                    osb[:, :w], osb[:, :w], ops[:, :w])
                nc.sync.dma_start(
                    out=out_r[:, ct, i0:i0 + w], in_=osb[:, :w])

            # ---------------- attention main loop ----------------
            pending = None
            for ib, (i0, w) in enumerate(IBLOCKS):
                isl = slice(i0, i0 + w)
                num1 = nump.tile([D + 1, IBLK], F32, tag="num", name="num1")
                num2 = nump.tile([D + 1, IBLK], F32, tag="num", name="num2")
                prev_et = None
                # num consumes et with a 2-iteration lag so the logits for
                # jt+1 are never queued behind matmuls that wait on a fresh
                # exp result (PE stays an exp ahead of Act)
                et_hist = [None, None]
                for jt in range(NJT + 2):
                    if ib == 0 and jt % 2 == 1 and (jt + 3) // 2 < NCH:
                        emit_proj((jt + 3) // 2)
                        emit_transposes((jt + 3) // 2)
                    if jt < NJT:
                        j0 = jt * P
                        lt = ltp.tile([P, 2, IBLK], F32, tag="lt", name="lt")
                        et = ebuf.tile([P, 2, IBLK], BF16, tag="et", name="et")
                        if ib == 0 and jt == 0:
                            # first logits/exp in 256-col halves: the second
                            # half's projections (chunk 1) land later than
                            # chunk 0's, so don't gate the first exp on them
                            for h0 in (0, 256):
                                hsl = slice(h0, h0 + 256)
                                nc.tensor.matmul(
                                    lt[:, 0, hsl], lhsT=g_sb[:, j0:j0 + P],
                                    rhs=f1h_sb[0:D, h0:h0 + 256],
                                    start=True, stop=True)
                                nc.tensor.matmul(
                                    lt[:, 1, hsl], lhsT=g_sb[:, j0:j0 + P],
                                    rhs=f2_sb[:, h0:h0 + 256],
                                    start=True, stop=True)
                                nc.scalar.activation(
                                    out=et[:, :, hsl], in_=lt[:, :, hsl],
                                    func=AF.Exp, bias=shiftb[:, 0:1],
                                    scale=1.0)
                        else:
                            nc.tensor.matmul(
                                lt[:, 0, :w], lhsT=g_sb[:, j0:j0 + P],
                                rhs=f1h_sb[0:D, isl], start=True, stop=True)
                            nc.tensor.matmul(
                                lt[:, 1, :w], lhsT=g_sb[:, j0:j0 + P],
                                rhs=f2_sb[:, isl], start=True, stop=True)
                            nc.scalar.activation(
                                out=et[:, :, :w], in_=lt[:, :, :w],
                                func=AF.Exp, bias=shiftb[:, 0:1], scale=1.0)
                    if ib == 0 and jt == 0:
                        emit_transposes(0)
                        emit_transposes(1)
                    if jt >= 2:
                        pj = jt - 2
                        st, sp = (pj == 0), (pj == NJT - 1)
                        nc.tensor.matmul(
                            num1[:, :w], lhsT=hT_sb[:, pj, :],
                            rhs=et_hist[0][:, 0, :w], start=st, stop=sp)
                        nc.tensor.matmul(
                            num2[:, :w], lhsT=hT_sb[:, pj, :],
                            rhs=et_hist[0][:, 1, :w], start=st, stop=sp)
                    if jt < NJT:
                        et_hist = [et_hist[1], et]
                    else:
                        et_hist = [et_hist[1], None]
                    if pending is not None and jt in (6, 8, 10, 12):
                        emit_out_ct(pending, (jt - 6) // 2)
                        if jt == 12:
                            pending = None
                rcp1 = rcpp.tile([1, IBLK], F32R, tag="rcp", name="rcp1")
                rcp2 = rcpp.tile([1, IBLK], F32R, tag="rcp", name="rcp2")
                with nc.allow_low_precision(
                        reason="softmax denominator reciprocal in f32r"):
                    nc.vector.reciprocal(rcp1[0:1, :w], num1[D:D + 1, :w])
                    nc.vector.reciprocal(rcp2[0:1, :w], num2[D:D + 1, :w])
                rb1 = scratch.tile([P, IBLK], F32, tag="sc", name="rb1")
                nc.tensor.matmul(
                    rb1[0:D, :w], lhsT=sel2[0:1, 0:D], rhs=rcp1[:, :w],
                    start=True, stop=True)
                rb2 = scratch.tile([P, IBLK], F32, tag="sc", name="rb2")
                nc.tensor.matmul(
                    rb2[0:D, :w], lhsT=sel2[0:1, 0:D], rhs=rcp2[:, :w],
                    start=True, stop=True)
                ea = eap.tile([P, IBLK], F32R, tag="ea", name="ea")
                nc.vector.tensor_copy(out=ea[0:D, :w], in_=num1[0:D, :w])
                nc.vector.tensor_copy(out=ea[D:P, :w], in_=num2[0:D, :w])
                nc.vector.tensor_mul(ea[0:D, :w], ea[0:D, :w], rb1[0:D, :w])
                nc.vector.tensor_mul(ea[D:P, :w], ea[D:P, :w], rb2[0:D, :w])
                pending = emit_out_start(ea, i0, w)
            for ct in range(NCT):
                emit_out_ct(pending, ct)

    nc.compile()
    return nc


_NC_CACHE = None


def _get_nc():
    global _NC_CACHE
    if _NC_CACHE is None:
        _NC_CACHE = build_program()
    return _NC_CACHE


def _run(inputs, trace=False, trace_cores=None):
    from concourse.bass_utils import run_bass_kernel_spmd

    import ml_dtypes
    g = {k: np.ascontiguousarray(np.asarray(v, dtype=np.float32))
         for k, v in inputs.items()}
    x = g["x"].reshape(BS, C, N)
    y = g["y"].reshape(BS, C, N)
    x16 = np.ascontiguousarray(x.astype(ml_dtypes.bfloat16))
    y16 = np.ascontiguousarray(y.astype(ml_dtypes.bfloat16))

    def core_inputs(b, s):
        def sel(a0, a1):
            return a0 if s == 0 else a1

        gate1 = float(np.asarray(sel(g["alpha"], g["gamma"])).reshape(-1)[0])
        gate2 = float(np.asarray(sel(g["beta"], g["sigma"])).reshape(-1)[0])
        Wcat = np.concatenate(
            [sel(g["Wg1"], g["Wg2"]).T, g["Wf1"].T,
             sel(g["Wh1"], g["Wh2"]).T, g["Wf2"].T], axis=1)   # [C, 256]
        WvS = np.concatenate(
            [gate1 * sel(g["Wv11"], g["Wv12"]).T,
             gate2 * sel(g["Wv21"], g["Wv22"]).T], axis=0)     # [128, C]
        cv = (gate1 * sel(g["bv11"], g["bv12"])
              + gate2 * sel(g["bv21"], g["bv22"]))             # [C]
        smalls = np.zeros((P, 8), np.float32)
        smalls[0:D, 0] = g["bf1"]
        smalls[D:P, 0] = sel(g["bh1"], g["bh2"])
        smalls[0:D, 1] = g["bf2"]
        smalls[0:D, 2] = sel(g["bg1"], g["bg2"])
        smalls[:, 4:8] = cv.reshape(NCT, P).T
        sel2 = np.zeros((2, P), np.float32)
        sel2[0, 0:D] = 1.0
        sel2[1, D:P] = 1.0
        return {
            "xin": np.ascontiguousarray(x16[b]),
            "yin": np.ascontiguousarray(y16[b]),
            "zin": np.ascontiguousarray(sel(x16, y16)[b]),
            "Wcat": np.ascontiguousarray(Wcat.astype(ml_dtypes.bfloat16)),
            "WvS": np.ascontiguousarray(WvS),
            "smalls": smalls,
            "sel": sel2,
        }

    in_maps = [core_inputs(core // 2, core % 2) for core in range(8)]
    res = run_bass_kernel_spmd(
        _get_nc(), in_maps, core_ids=list(range(8)), trace=trace,
        trace_cores=trace_cores)
    outs = [r["out"] for r in res.results]
    x_out = np.stack([outs[2 * b] for b in range(BS)]).reshape(BS, C, H, W)
    y_out = np.stack([outs[2 * b + 1] for b in range(BS)]).reshape(BS, C, H, W)
    return (x_out, y_out), res


def kernel(**inputs):
    out, _ = _run(inputs)
    return out
